# revision 10
# baseline (speedup 1.0000x reference)
"""HMM posterior kernel for Trainium2 (8 NeuronCores, SPMD data-parallel over batch).

Math: in the reference,
    ln_fs + ln_bs = (cs + ln_pi + t*ln_diag)
                  + (ln_pi + ln_emis[T-1] + (total - cs) + (T-1-t)*ln_diag)
                  = 2*ln_pi + ln_emis[:,T-1,:] + total + (T-1)*ln_diag
The cumsum terms cancel exactly, so the pre-normalization log_gamma is
independent of t, and so is its logsumexp over k.  The output is a [B, K]
tensor broadcast over the T axis.  With S1 = sum_t x, S2 = sum_t x^2,
xl = x[T-1], e = exp(-2*ls):

    g[b,k] = A[b]*e[k] + Bc[b]*(e*mu)[k] + 1*hm1[k] + kc[k]   (+ const, which
                                            cancels in the logsumexp)
    A  = -0.5*(S2 + xl^2)       hm1 = -0.5*(T+1)*mu^2*e
    Bc = S1 + xl                kc  = -(T+1)*ls + 2*pi + (T-1)*di

so g accumulates in PSUM from four rank-1/rank-4 matmuls whose operands
all live on partition 0 (or partitions 0-3 for the params matmul).  The
kc coefficients and the row-replication selectors are host-provided
constants.  Stats come from a ones-matmul over the partition axis.

Each core handles B/8 = 4 batch rows.  The kernel is output-write bound
(memory regime): the [4, T, K] shard is written in bf16 (final values
only; all compute in f32; ~2e-3 rel rounding vs the 2e-2 gate) and
widened to f32 on the host.  Each row's gn is replicated x4 in SBUF so
the stride-0 broadcast DMAs move 4 KB descriptors, and the 4 row writes
alternate between the two HWDGE rings (sync + scalar).
"""

import numpy as np

B, T, K = 32, 2048, 512
NCORES = 8
BS = B // NCORES  # 4 batch rows per core
W = 16            # t = p*W + w layout for the obvs stats pass
RJ = T // 128     # 16 t-rows per partition per batch row
REP = 4           # replication factor -> 4 KB DMA descriptors
LOG_2PI = float(np.log(2.0 * np.pi))
C = 0.5 * LOG_2PI

_BUILT = {}


def _build_nc(split_waits=True):
    key = ("nc", split_waits)
    if key in _BUILT:
        return _BUILT[key]

    from concourse import bass, tile
    import concourse.mybir as mybir

    f32 = mybir.dt.float32
    bf16 = mybir.dt.bfloat16
    AF = mybir.ActivationFunctionType
    ALU = mybir.AluOpType
    X = mybir.AxisListType.X

    nc = bass.Bass()
    obvs = nc.declare_dram_parameter("obvs", [BS, T], f32, isOutput=False)
    par4 = nc.declare_dram_parameter("par4", [4, K], f32, isOutput=False)
    cst4 = nc.declare_dram_parameter("cst4", [4, BS], f32, isOutput=False)
    selc = nc.declare_dram_parameter("selc", [BS, BS * 128], bf16, isOutput=False)
    out = nc.declare_dram_parameter("out", [BS, T, K], bf16, isOutput=True)

    with tile.TileContext(nc) as tc:
        with (
            tc.tile_pool(name="sbuf", bufs=1) as pool,
            tc.tile_pool(name="psum", bufs=1, space="PSUM") as psum,
        ):
            # ---- loads first.  sync ring: obvs, x_last row, params
            # partition-major; scalar ring: params row-major (mu, ls only),
            # kc coefficients, row selectors. ----
            obsq = pool.tile([128, 2, BS, W], f32)
            nc.sync.dma_start(
                out=obsq[:, 0], in_=obvs[:].rearrange("b (p w) -> p b w", w=W)
            )
            xlr = pool.tile([1, BS], f32)
            nc.sync.dma_start(
                out=xlr[:], in_=obvs[:, T - 1 : T].rearrange("b one -> one b")
            )
            pp = pool.tile([4, K], f32)
            nc.sync.dma_start(out=pp[:], in_=par4[:])
            pc = pool.tile([1, 2 * K], f32)
            nc.scalar.dma_start(
                out=pc[:], in_=par4[0:2].rearrange("q k -> (q k)").unsqueeze(0)
            )
            mu_r = pc[0:1, 0 * K : 1 * K]
            ls_r = pc[0:1, 1 * K : 2 * K]
            cc4 = pool.tile([4, BS], f32)
            nc.scalar.dma_start(out=cc4[:], in_=cst4[:])
            sel4 = pool.tile([BS, BS * 128], bf16)
            nc.scalar.dma_start(out=sel4[:], in_=selc[:])
            onesr = pool.tile([1, BS], f32)
            nc.vector.memset(onesr[:], 1.0)

            # ---- param-side rows on ACT + DVE ----
            er = pool.tile([1, K], f32)
            nc.scalar.activation(er[:], ls_r, AF.Exp, scale=-2.0)
            mu2r = pool.tile([1, K], f32)
            nc.scalar.activation(mu2r[:], mu_r, AF.Square)
            r1 = pool.tile([1, K], f32)
            nc.vector.tensor_mul(r1[:], er[:], mu_r)
            hm1 = pool.tile([1, K], f32)
            nc.vector.scalar_tensor_tensor(
                out=hm1[:], in0=mu2r[:], scalar=-0.5 * (float(T) + 1.0),
                in1=er[:], op0=ALU.mult, op1=ALU.mult,
            )

            # ---- batch stats: S1, S2 via Square + ones-matmul + one reduce ----
            ones_col = pool.tile([128, 1], f32)
            nc.vector.memset(ones_col[:], 1.0)
            nc.scalar.activation(obsq[:, 1], obsq[:, 0], AF.Square)
            ps_s = psum.tile([1, 2 * BS * W], f32)
            nc.tensor.matmul(
                ps_s[:],
                lhsT=ones_col[:],
                rhs=obsq[:].rearrange("p a b w -> p (a b w)"),
                start=True,
                stop=True,
            )
            srow = pool.tile([1, 2 * BS], f32)
            nc.vector.reduce_sum(
                srow[:].unsqueeze(2),
                ps_s[:].rearrange("o (ab w) -> o ab w", w=W),
                axis=X,
            )
            # A = -0.5*(S2 + xl^2) ; Bc = S1 + xl   (all [1, BS] on p0)
            xl2h = pool.tile([1, BS], f32)
            nc.vector.scalar_tensor_tensor(
                out=xl2h[:], in0=xlr[:], scalar=-0.5, in1=xlr[:],
                op0=ALU.mult, op1=ALU.mult,
            )
            a_r = pool.tile([1, BS], f32)
            nc.vector.scalar_tensor_tensor(
                out=a_r[:], in0=srow[0:1, BS : 2 * BS], scalar=-0.5,
                in1=xl2h[:], op0=ALU.mult, op1=ALU.add,
            )
            b_r = pool.tile([1, BS], f32)
            nc.vector.tensor_add(b_r[:], srow[0:1, 0:BS], xlr[:])

            # ---- g accumulates in PSUM from four small matmuls ----
            g_ps = psum.tile([BS, K], f32, tag="gps", name="gps")
            nc.tensor.matmul(
                g_ps[:], lhsT=cc4[:], rhs=pp[:], start=True, stop=False
            )
            nc.tensor.matmul(
                g_ps[:], lhsT=onesr[:], rhs=hm1[:], start=False, stop=False
            )
            nc.tensor.matmul(
                g_ps[:], lhsT=a_r[:], rhs=er[:], start=False, stop=False
            )
            nc.tensor.matmul(
                g_ps[:], lhsT=b_r[:], rhs=r1[:], start=False, stop=True
            )

            # ---- logsumexp over k (fused), normalize, cast to bf16 ----
            negm = pool.tile([BS, 1], f32)
            nc.vector.reduce_max(negm[:], g_ps[:], axis=X, negate=True)
            et = pool.tile([BS, K], f32)
            s = pool.tile([BS, 1], f32)
            nc.scalar.activation(
                et[:], g_ps[:], AF.Exp, bias=negm[:], accum_out=s[:]
            )
            nls = pool.tile([BS, 1], f32)
            nc.scalar.activation(nls[:], s[:], AF.Ln)
            gn = pool.tile([BS, K], bf16)
            nc.vector.tensor_scalar(
                out=gn[:],
                in0=g_ps[:],
                scalar1=negm[:],
                scalar2=nls[:],
                op0=ALU.add,
                op1=ALU.subtract,
            )

            # ---- broadcast write: out[b, t, :] = gn[b, :] for all t ----
            # PE bf16 matmul replicates row b across 128 partitions; four
            # casts (2 DVE + 2 ACT) build a [4K] block per partition (4 KB
            # DMA descriptors); one 2 MB stride-0 DMA per row, alternating
            # rings.
            bt4 = pool.tile([128, BS, REP * K], bf16)
            for b in range(BS):
                psB = psum.tile([128, K], f32, tag=f"psb{b}", name=f"psb{b}")
                nc.tensor.matmul(
                    psB[:],
                    lhsT=sel4[:, b * 128 : (b + 1) * 128],
                    rhs=gn[:],
                    start=True,
                    stop=True,
                )
                nc.vector.tensor_copy(bt4[:, b, 0 * K : 1 * K], psB[:])
                nc.scalar.copy(bt4[:, b, 1 * K : 2 * K], psB[:])
                nc.vector.tensor_copy(bt4[:, b, 2 * K : 3 * K], psB[:])
                nc.scalar.copy(bt4[:, b, 3 * K : 4 * K], psB[:])
                eng = nc.sync if b % 2 == 0 else nc.scalar
                eng.dma_start(
                    out=out[b].rearrange(
                        "(p j r) k -> p j (r k)", j=RJ // REP, r=REP
                    ),
                    in_=bt4[:, b, :]
                    .unsqueeze(1)
                    .broadcast_to([128, RJ // REP, REP * K]),
                )

    if split_waits:
        _split_multi_waits(nc, mybir)
    _BUILT[key] = nc
    return nc


def _split_multi_waits(nc, mybir):
    """This walrus build allows at most ONE sync wait per instruction.  Split
    any instruction with N>1 waits into N-1 single-wait NoOps on the same
    engine (executed immediately before it by the same sequencer) plus the
    original instruction carrying the final wait."""
    for fn in nc.m.functions:
        for blk in fn.blocks:
            new_insts = []
            for inst in blk.instructions:
                si = inst.sync_info
                if si is not None and len(si.on_wait) > 1:
                    waits = list(si.on_wait)
                    for i, w in enumerate(waits[:-1]):
                        new_insts.append(
                            mybir.InstNoOp(
                                name=f"{inst.name}-sw{i}",
                                engine=inst.engine,
                                sync_info=mybir.SyncInfo(
                                    on_wait=[w], on_update=[]
                                ),
                                bass_nofuse=True,
                            )
                        )
                    inst.sync_info = mybir.SyncInfo(
                        on_wait=[waits[-1]], on_update=list(si.on_update)
                    )
                new_insts.append(inst)
            blk.instructions = new_insts


def _host_constants():
    # kc coefficients for [mu, ls, pi, di] rows, replicated across batch cols
    coef = np.array([0.0, -(float(T) + 1.0), 2.0, float(T - 1)], dtype=np.float32)
    cst4 = np.repeat(coef[:, None], BS, axis=1)  # [4, BS]
    # row-replication selectors: selc[:, b*128:(b+1)*128] = e_b x ones(128)
    import ml_dtypes

    selc = np.zeros((BS, BS * 128), dtype=ml_dtypes.bfloat16)
    for b in range(BS):
        selc[b, b * 128 : (b + 1) * 128] = 1.0
    return np.ascontiguousarray(cst4), np.ascontiguousarray(selc)


def _run(inputs, trace=False, trace_kwargs=None):
    from concourse.bass_utils import run_bass_kernel_spmd

    nc = _build_nc()
    obvs = np.ascontiguousarray(np.asarray(inputs["obvs"], dtype=np.float32))
    par4 = np.ascontiguousarray(
        np.stack(
            [
                np.asarray(inputs["mu"], dtype=np.float32),
                np.asarray(inputs["log_sigma"], dtype=np.float32),
                np.asarray(inputs["ln_pi"], dtype=np.float32),
                np.asarray(inputs["ln_diag"], dtype=np.float32),
            ]
        )
    )
    cst4, selc = _host_constants()
    in_maps = [
        {
            "obvs": obvs[c * BS : (c + 1) * BS],
            "par4": par4,
            "cst4": cst4,
            "selc": selc,
        }
        for c in range(NCORES)
    ]
    kw = {}
    if trace:
        kw["trace"] = True
        if trace_kwargs:
            kw["trace_kwargs"] = trace_kwargs
    res = run_bass_kernel_spmd(nc, in_maps, list(range(NCORES)), **kw)
    full = np.empty((B, T, K), dtype=np.float32)
    for c in range(NCORES):
        full[c * BS : (c + 1) * BS] = np.asarray(res.results[c]["out"]).astype(
            np.float32
        )
    return full, res


def kernel(**inputs) -> np.ndarray:
    full, _ = _run(inputs, trace=False)
    return full


# revision 11
# speedup vs baseline: 1.1589x; 1.1589x over previous
"""HMM posterior kernel for Trainium2 (8 NeuronCores, SPMD data-parallel over batch).

Math: in the reference,
    ln_fs + ln_bs = (cs + ln_pi + t*ln_diag)
                  + (ln_pi + ln_emis[T-1] + (total - cs) + (T-1-t)*ln_diag)
                  = 2*ln_pi + ln_emis[:,T-1,:] + total + (T-1)*ln_diag
The cumsum terms cancel exactly, so the pre-normalization log_gamma is
independent of t, and so is its logsumexp over k.  The output is a [B, K]
tensor broadcast over the T axis.  With S1 = sum_t x, S2 = sum_t x^2,
xl = x[T-1], e = exp(-2*ls) (the -(T+1)*C constant cancels in the
logsumexp):

    g[b,k] = A[b]*eh[k] + Bc[b]*r1[k] + 1*r2[k]            (rank-3)
    A  = S2 + xl^2          eh = -0.5*e
    Bc = S1 + xl            r1 = e*mu
                            r2 = -0.5*(T+1)*mu^2*e + kc
    kc = -(T+1)*ls + 2*pi + (T-1)*di

so g is ONE bf16 PE matmul of CC[96, BS] (rows 0/32/64 = A/Bc/1) against
RR[96, K] (rows 0/32/64 = eh/r1/r2); bf16 operand rounding (~0.4%)
is far inside the 2e-2 gate.  Stats come from a ones-matmul over the
partition axis + one DVE reduce; params arrive as one concatenated
[4, K] DRAM tensor; the row-replication selectors are a host constant.

Each core handles B/8 = 4 batch rows.  The kernel is output-write bound
(memory regime): the [4, T, K] shard is written in bf16 and widened to
f32 on the host.  Each row's gn is replicated x2 in SBUF (2 KB DMA
descriptors, ~HBM line rate) and the 4 row writes alternate between the
two HWDGE rings (sync + scalar), which measures ~98% SDMA utilization.
"""

import numpy as np

B, T, K = 32, 2048, 512
NCORES = 8
BS = B // NCORES  # 4 batch rows per core
W = 16            # t = p*W + w layout for the obvs stats pass
RJ = T // 128     # 16 t-rows per partition per batch row
REP = 2           # replication factor -> 2 KB DMA descriptors
LOG_2PI = float(np.log(2.0 * np.pi))
C = 0.5 * LOG_2PI

_BUILT = {}


def _build_nc(split_waits=True):
    key = ("nc", split_waits)
    if key in _BUILT:
        return _BUILT[key]

    from concourse import bass, tile
    import concourse.mybir as mybir

    f32 = mybir.dt.float32
    bf16 = mybir.dt.bfloat16
    AF = mybir.ActivationFunctionType
    ALU = mybir.AluOpType
    X = mybir.AxisListType.X

    nc = bass.Bass()
    obvs = nc.declare_dram_parameter("obvs", [BS, T], f32, isOutput=False)
    par4 = nc.declare_dram_parameter("par4", [4, K], f32, isOutput=False)
    selc = nc.declare_dram_parameter("selc", [BS, BS * 128], bf16, isOutput=False)
    out = nc.declare_dram_parameter("out", [BS, T, K], bf16, isOutput=True)

    with tile.TileContext(nc) as tc:
        with (
            tc.tile_pool(name="sbuf", bufs=1) as pool,
            tc.tile_pool(name="psum", bufs=1, space="PSUM") as psum,
        ):
            # ---- all loads on the sync ring so the scalar (ACT) ring is
            # free: its activation-table load then runs during the DMA
            # flight instead of after it. ----
            pc = pool.tile([1, 4 * K], f32)
            nc.sync.dma_start(
                out=pc[:], in_=par4[:].rearrange("q k -> (q k)").unsqueeze(0)
            )
            obsq = pool.tile([128, 2, BS, W], f32)
            nc.sync.dma_start(
                out=obsq[:, 0], in_=obvs[:].rearrange("b (p w) -> p b w", w=W)
            )
            xlr = pool.tile([1, BS], f32)
            nc.sync.dma_start(
                out=xlr[:], in_=obvs[:, T - 1 : T].rearrange("b one -> one b")
            )
            sel4 = pool.tile([BS, BS * 128], bf16)
            nc.sync.dma_start(out=sel4[:], in_=selc[:])
            mu_r = pc[0:1, 0 * K : 1 * K]
            ls_r = pc[0:1, 1 * K : 2 * K]
            pi_r = pc[0:1, 2 * K : 3 * K]
            di_r = pc[0:1, 3 * K : 4 * K]

            # ---- zero-padded bf16 matmul operands (memsets off the path) ----
            CC = pool.tile([96, BS], bf16)
            nc.vector.memset(CC[:], 0.0)
            nc.vector.memset(CC[64:65, :], 1.0)
            RR = pool.tile([96, K], bf16)
            nc.vector.memset(RR[:], 0.0)
            ones_col = pool.tile([128, 1], f32)
            nc.vector.memset(ones_col[:], 1.0)

            # ---- ACT chain: e = exp(-2*ls), x^2, mu^2, kc1 = -(T+1)*ls,
            # eh = -0.5*e -> RR row 0 (bf16 cast on output) ----
            er = pool.tile([1, K], f32)
            nc.scalar.activation(er[:], ls_r, AF.Exp, scale=-2.0)
            nc.scalar.activation(obsq[:, 1], obsq[:, 0], AF.Square)
            mu2r = pool.tile([1, K], f32)
            nc.scalar.activation(mu2r[:], mu_r, AF.Square)
            kc1 = pool.tile([1, K], f32)
            nc.scalar.activation(kc1[:], ls_r, AF.Copy, scale=-(float(T) + 1.0))
            nc.scalar.activation(RR[0:1, :], er[:], AF.Copy, scale=-0.5)

            # ---- stats matmul (PE) ----
            ps_s = psum.tile([1, 2 * BS * W], f32)
            nc.tensor.matmul(
                ps_s[:],
                lhsT=ones_col[:],
                rhs=obsq[:].rearrange("p a b w -> p (a b w)"),
                start=True,
                stop=True,
            )

            # ---- DVE chain (in program = execution order) ----
            # r1 = e*mu -> RR row 32
            nc.vector.tensor_mul(RR[32:33, :], er[:], mu_r)
            xl2r = pool.tile([1, BS], f32)
            nc.vector.tensor_mul(xl2r[:], xlr[:], xlr[:])
            srow = pool.tile([1, 2 * BS], f32)
            nc.vector.reduce_sum(
                srow[:].unsqueeze(2),
                ps_s[:].rearrange("o (ab w) -> o ab w", w=W),
                axis=X,
            )
            # A = S2 + xl^2 -> CC row 0 ; Bc = S1 + xl -> CC row 32
            nc.vector.tensor_add(CC[0:1, :], srow[0:1, BS : 2 * BS], xl2r[:])
            nc.vector.tensor_add(CC[32:33, :], srow[0:1, 0:BS], xlr[:])
            # kc = kc1 + 2*pi + (T-1)*di
            kc2 = pool.tile([1, K], f32)
            nc.vector.scalar_tensor_tensor(
                out=kc2[:], in0=di_r, scalar=float(T - 1), in1=kc1[:],
                op0=ALU.mult, op1=ALU.add,
            )
            kcr = pool.tile([1, K], f32)
            nc.vector.scalar_tensor_tensor(
                out=kcr[:], in0=pi_r, scalar=2.0, in1=kc2[:],
                op0=ALU.mult, op1=ALU.add,
            )
            # r2 = -0.5*(T+1)*mu^2*e + kc -> RR row 64
            hm1 = pool.tile([1, K], f32)
            nc.vector.scalar_tensor_tensor(
                out=hm1[:], in0=mu2r[:], scalar=-0.5 * (float(T) + 1.0),
                in1=er[:], op0=ALU.mult, op1=ALU.mult,
            )
            nc.vector.tensor_add(RR[64:65, :], hm1[:], kcr[:])

            # ---- g = CC^T @ RR (one bf16 PE matmul), then logsumexp ----
            g_ps = psum.tile([BS, K], f32, tag="gps", name="gps")
            nc.tensor.matmul(
                g_ps[:], lhsT=CC[:], rhs=RR[:], start=True, stop=True
            )
            negm = pool.tile([BS, 1], f32)
            nc.vector.reduce_max(negm[:], g_ps[:], axis=X, negate=True)
            et = pool.tile([BS, K], f32)
            s = pool.tile([BS, 1], f32)
            nc.scalar.activation(
                et[:], g_ps[:], AF.Exp, bias=negm[:], accum_out=s[:]
            )
            nls = pool.tile([BS, 1], f32)
            nc.scalar.activation(nls[:], s[:], AF.Ln)
            gn = pool.tile([BS, K], bf16)
            nc.vector.tensor_scalar(
                out=gn[:],
                in0=g_ps[:],
                scalar1=negm[:],
                scalar2=nls[:],
                op0=ALU.add,
                op1=ALU.subtract,
            )

            # ---- broadcast write: out[b, t, :] = gn[b, :] for all t ----
            # PE bf16 matmul replicates row b across 128 partitions; one DVE
            # + one ACT cast build the doubled [2K] block per partition
            # (2 KB descriptors); one 2 MB stride-0 DMA per row, alternating
            # rings.
            bt2 = pool.tile([128, BS, REP * K], bf16)
            for b in range(BS):
                psB = psum.tile([128, K], f32, tag=f"psb{b}", name=f"psb{b}")
                nc.tensor.matmul(
                    psB[:],
                    lhsT=sel4[:, b * 128 : (b + 1) * 128],
                    rhs=gn[:],
                    start=True,
                    stop=True,
                )
                nc.vector.tensor_copy(bt2[:, b, 0 * K : 1 * K], psB[:])
                nc.scalar.copy(bt2[:, b, 1 * K : 2 * K], psB[:])
                eng = nc.sync if b % 2 == 0 else nc.scalar
                eng.dma_start(
                    out=out[b].rearrange(
                        "(p j r) k -> p j (r k)", j=RJ // REP, r=REP
                    ),
                    in_=bt2[:, b, :]
                    .unsqueeze(1)
                    .broadcast_to([128, RJ // REP, REP * K]),
                )

    if split_waits:
        _split_multi_waits(nc, mybir)
    _BUILT[key] = nc
    return nc


def _split_multi_waits(nc, mybir):
    """This walrus build allows at most ONE sync wait per instruction.  Split
    any instruction with N>1 waits into N-1 single-wait NoOps on the same
    engine (executed immediately before it by the same sequencer) plus the
    original instruction carrying the final wait."""
    for fn in nc.m.functions:
        for blk in fn.blocks:
            new_insts = []
            for inst in blk.instructions:
                si = inst.sync_info
                if si is not None and len(si.on_wait) > 1:
                    waits = list(si.on_wait)
                    for i, w in enumerate(waits[:-1]):
                        new_insts.append(
                            mybir.InstNoOp(
                                name=f"{inst.name}-sw{i}",
                                engine=inst.engine,
                                sync_info=mybir.SyncInfo(
                                    on_wait=[w], on_update=[]
                                ),
                                bass_nofuse=True,
                            )
                        )
                    inst.sync_info = mybir.SyncInfo(
                        on_wait=[waits[-1]], on_update=list(si.on_update)
                    )
                new_insts.append(inst)
            blk.instructions = new_insts


def _host_constants():
    # row-replication selectors: selc[:, b*128:(b+1)*128] = e_b x ones(128)
    import ml_dtypes

    selc = np.zeros((BS, BS * 128), dtype=ml_dtypes.bfloat16)
    for b in range(BS):
        selc[b, b * 128 : (b + 1) * 128] = 1.0
    return np.ascontiguousarray(selc)


def _run(inputs, trace=False, trace_kwargs=None):
    from concourse.bass_utils import run_bass_kernel_spmd

    nc = _build_nc()
    obvs = np.ascontiguousarray(np.asarray(inputs["obvs"], dtype=np.float32))
    par4 = np.ascontiguousarray(
        np.stack(
            [
                np.asarray(inputs["mu"], dtype=np.float32),
                np.asarray(inputs["log_sigma"], dtype=np.float32),
                np.asarray(inputs["ln_pi"], dtype=np.float32),
                np.asarray(inputs["ln_diag"], dtype=np.float32),
            ]
        )
    )
    selc = _host_constants()
    in_maps = [
        {"obvs": obvs[c * BS : (c + 1) * BS], "par4": par4, "selc": selc}
        for c in range(NCORES)
    ]
    kw = {}
    if trace:
        kw["trace"] = True
        if trace_kwargs:
            kw["trace_kwargs"] = trace_kwargs
    res = run_bass_kernel_spmd(nc, in_maps, list(range(NCORES)), **kw)
    full = np.empty((B, T, K), dtype=np.float32)
    for c in range(NCORES):
        full[c * BS : (c + 1) * BS] = np.asarray(res.results[c]["out"]).astype(
            np.float32
        )
    return full, res


def kernel(**inputs) -> np.ndarray:
    full, _ = _run(inputs, trace=False)
    return full
